# revision 1
# baseline (speedup 1.0000x reference)
"""Trainium2 Bass kernel v2 for nn_Network_5772436046487 (gnn_message_passing).

Recurrence (T=50, B=8, N=50000, E=1.6M):
    v' = v + DT*(-v + bias + scatter_add(w * relu(v)[src], tgt) + x_t)/tau

Sharding: core c owns targets [6250c, 6250(c+1)), split into 8 target groups
(gc) of <=782 real nodes, padded to 896 each (CORE_PAD=7168, global padded
node space 57344).  Partition map p = 16*gc + 8*s + b where s = source half
(padded global position < or >= 28672) and b = batch.

Per step:
  relu -> r_own DRAM [B,7168] fp16 -> AllGather r_all [8, B*7168]
  -> 16 DMAs build gather table [128, 14336 pairs of fp16] (partition p
     holds r_b of half s; position j = (c-4s)*7168 + ppos)
  -> per chunk: ap_gather d=2 (idxA: stream (gc,0), idxB: stream (gc,1));
     in-place mul by parity+partition-masked fp8 weights (x256); strided
     pair-add -> PA/PB; tensor_tensor_scan(add,add) accumulates both
     streams' prefix sums into X[128, STREAM] fp32 (per-partition prefix of
     its own stream).
  -> 2 boundary ap_gathers sample X at per-target end slots; diff ->
     per-(gc,s,b) target sums D [128, 2*896]
  -> 4 PE matmuls with selA/selB [128,64] (weights 2^-8, undoing the fp8
     x256 scale) reduce s and map to update layout [64=(gc,b), 896]
  -> DVE update v = v*Ad + (psum + x'); DMA out.

All graph structure (streams sorted by local target, slot 0 dummy, boundary
positions) precomputed host-side in numpy.
"""

import os
import sys
import time

os.environ.setdefault("JAX_COMPILATION_CACHE_DIR", "/tmp/jax_cache_gnn")
os.environ.setdefault("JAX_PERSISTENT_CACHE_MIN_COMPILE_TIME_SECS", "2")
os.environ.setdefault("JAX_PERSISTENT_CACHE_MIN_ENTRY_SIZE_BYTES", "0")

for _p in ("/opt/trn_rl_repo", "/root/.axon_site/_ro/trn_rl_repo"):
    if os.path.isdir(_p) and _p not in sys.path:
        sys.path.insert(0, _p)

import numpy as np

N_NODES = 50000
N_EDGES = 1_600_000
T = 50
B = 8
DT = 0.02

NC = 8            # neuron cores
CORE_REAL = 6250  # real targets per core
NGC = 8           # target groups (gpsimd cores) per core
TPG = 896         # padded targets per group (7*128)
CORE_PAD = NGC * TPG      # 7168
TOT_PAD = NC * CORE_PAD   # 57344
HALF = TOT_PAD // 2       # 28672
NPAIR = HALF // 2         # 14336 fp16 pairs per table row

# real targets per group: 2x782 + 6x781 = 6250
GC_REAL = np.array([782, 782, 781, 781, 781, 781, 781, 781])
GC_START = np.concatenate([[0], np.cumsum(GC_REAL)[:-1]])

CH = 3168         # slots per edge chunk (16*198)
NCH = 4
STREAM = NCH * CH  # 13056 slots per (gc,s) stream (incl dummy slot 0)

W_SCALE = 256.0   # fp8 weight scale; undone by 2^-8 in sel matmuls

_CACHE = {}


def _wrap16(groups):
    """groups: [8, N] -> [128, N//16] int16, wrapped per 16 partitions."""
    G, N = groups.shape
    assert G == 8 and N % 16 == 0
    out = np.empty((128, N // 16), dtype=np.int16)
    for g in range(8):
        out[16 * g: 16 * g + 16, :] = (
            groups[g].reshape(N // 16, 16).T.astype(np.int16))
    return out


def _preprocess(x, bias, time_const, sign, syn_count, syn_strength,
                source_idx, target_idx):
    import ml_dtypes

    tau = np.maximum(time_const.astype(np.float64), DT)
    BC = DT / tau                                   # (N,) f64
    A = (1.0 - DT / tau).astype(np.float32)
    weight = (sign.astype(np.float64) * syn_count.astype(np.float64)
              * np.maximum(syn_strength.astype(np.float64), 0.0))

    src = source_idx.astype(np.int64)
    tgt = target_idx.astype(np.int64)

    # Balanced target->gc assignment: per core, assign targets to the 8
    # groups greedily by per-half in-degree so per-(gc,s) stream lengths
    # equalize (source half depends only on the source CORE, so degrees
    # are fixed independent of the assignment).  Count capped at 782.
    e_shalf = (src // CORE_REAL) // 4      # source half per edge
    deg = np.zeros((2, N_NODES), dtype=np.int64)
    for ss in range(2):
        deg[ss] = np.bincount(tgt[e_shalf == ss], minlength=N_NODES)
    node_gc = np.empty(N_NODES, dtype=np.int64)
    node_tl = np.empty(N_NODES, dtype=np.int64)
    for c in range(NC):
        nodes = np.arange(c * CORE_REAL, (c + 1) * CORE_REAL)
        order_d = nodes[np.argsort(-(deg[0, nodes] + deg[1, nodes]),
                                   kind="stable")]
        load = np.zeros((2, NGC), dtype=np.int64)
        cnt = np.zeros(NGC, dtype=np.int64)
        for n in order_d:
            d0, d1 = deg[0, n], deg[1, n]
            best_g, best_m = -1, None
            for g in range(NGC):
                if cnt[g] >= 782:
                    continue
                m = max(load[0, g] + d0, load[1, g] + d1)
                if best_m is None or m < best_m:
                    best_m, best_g = m, g
            node_gc[n] = best_g
            node_tl[n] = cnt[best_g]
            load[0, best_g] += d0
            load[1, best_g] += d1
            cnt[best_g] += 1

    def pad_pos(n):
        c = n // CORE_REAL
        gc = node_gc[n]
        tl = node_tl[n]
        return c, gc, tl, c * CORE_PAD + gc * TPG + tl

    s_c, s_gc, s_tl, s_pp = pad_pos(src)
    t_c, t_gc, t_tl, _ = pad_pos(tgt)
    s_half = s_pp // HALF                  # stream s
    s_hpos = s_pp % HALF
    wprime = weight * BC[tgt]              # f64, includes DT/tau[tgt]

    # sort edges by (target core, target gc, source half, local target)
    order = np.lexsort((t_tl, s_half, t_gc, t_c))
    t_c, t_gc, t_tl = t_c[order], t_gc[order], t_tl[order]
    s_half, s_hpos, wp = s_half[order], s_hpos[order], wprime[order]

    key = (t_c * NGC + t_gc) * 2 + s_half
    starts = np.searchsorted(key, np.arange(NC * NGC * 2), side="left")
    ends = np.searchsorted(key, np.arange(NC * NGC * 2), side="right")
    maxlen = int((ends - starts).max())
    assert maxlen + 1 <= STREAM, f"stream overflow: {maxlen + 1} > {STREAM}"

    per_core = []
    for c in range(NC):
        idx_streams = np.zeros((2, 8, STREAM), dtype=np.int16)
        wq = np.zeros((2, 128, 2 * STREAM), dtype=np.float32)
        bpos = np.zeros((2, 8, TPG), dtype=np.int64)
        for gg in range(NGC):
            for ss in range(2):
                k = (c * NGC + gg) * 2 + ss
                a, b_ = int(starts[k]), int(ends[k])
                n = b_ - a
                hp = s_hpos[a:b_]
                idx_streams[ss, gg, 1: n + 1] = hp // 2
                par = (hp % 2).astype(np.int64)
                slots = np.arange(1, n + 1)
                w_sc = (wp[a:b_] * W_SCALE).astype(np.float32)
                # weights at free position 2*slot + parity, on partitions
                # p = 16*gg + 8*ss + b for all b
                rowbase = 16 * gg + 8 * ss
                flat = 2 * slots + par
                for bb in range(B):
                    wq[ss, rowbase + bb, flat] = w_sc
                # boundary: slot of last edge with t_local <= t
                cnt = np.searchsorted(t_tl[a:b_], np.arange(TPG),
                                      side="right")
                bpos[ss, gg] = cnt  # slot index (dummy at 0 shifts by +0)
        idxA = _wrap16(idx_streams[0])
        idxB = _wrap16(idx_streams[1])
        bidxA = _wrap16(bpos[0][:, :784])
        bidxB = _wrap16(bpos[1][:, :784])
        wqA = wq[0].astype(ml_dtypes.float8_e4m3)
        wqB = wq[1].astype(ml_dtypes.float8_e4m3)

        # sel matrices [128, 64]: p=(gc,s,b) -> q=(gc,b), weight 1/W_SCALE
        selA = np.zeros((128, 64), dtype=np.float32)
        selB = np.zeros((128, 64), dtype=np.float32)
        for gg in range(NGC):
            for bb in range(B):
                selA[16 * gg + bb, 8 * bb + gg] = 1.0 / W_SCALE
                selB[16 * gg + 8 + bb, 8 * bb + gg] = 1.0 / W_SCALE

        # node constants in update layout [64=(b,gc), TPG]
        n0 = c * CORE_REAL
        nodes = np.arange(n0, n0 + CORE_REAL)
        gcs = node_gc[nodes]
        tls = node_tl[nodes]
        Ad = np.zeros((64, TPG), dtype=np.float32)
        v0 = np.zeros((64, TPG), dtype=np.float32)
        Tl = x.shape[0]
        xp = np.zeros((Tl, 64, TPG), dtype=np.float16)
        for bb in range(B):
            q = 8 * bb + gcs
            Ad[q, tls] = A[nodes]
            v0[q, tls] = bias[nodes]
            xp[:, q, tls] = (
                BC[nodes][None, :]
                * (x[:, bb, nodes].astype(np.float64) + bias[nodes][None, :])
            ).astype(np.float16)

        per_core.append(dict(
            idxA=idxA, idxB=idxB, bidxA=bidxA, bidxB=bidxB,
            wqA=wqA, wqB=wqB, selA=selA, selB=selB,
            Ad=Ad, v0=v0, xp=np.ascontiguousarray(xp),
        ))
    _CACHE["maps"] = (node_gc, node_tl)
    return per_core


def _build(T_steps):
    import concourse.bacc as bacc
    import concourse.mybir as mybir
    import concourse.tile as tile

    dt = mybir.dt
    AF = mybir.ActivationFunctionType
    OP = mybir.AluOpType
    nc = bacc.Bacc("TRN2", target_bir_lowering=False, debug=False,
                   num_devices=NC)

    idxA_d = nc.dram_tensor("idxA", [128, STREAM // 16], dt.int16,
                            kind="ExternalInput")
    idxB_d = nc.dram_tensor("idxB", [128, STREAM // 16], dt.int16,
                            kind="ExternalInput")
    bidxA_d = nc.dram_tensor("bidxA", [128, 784 // 16], dt.int16,
                             kind="ExternalInput")
    bidxB_d = nc.dram_tensor("bidxB", [128, 784 // 16], dt.int16,
                             kind="ExternalInput")
    wqA_d = nc.dram_tensor("wqA", [128, 2 * STREAM], dt.float8e4,
                           kind="ExternalInput")
    wqB_d = nc.dram_tensor("wqB", [128, 2 * STREAM], dt.float8e4,
                           kind="ExternalInput")
    selA_d = nc.dram_tensor("selA", [128, 64], dt.float32,
                            kind="ExternalInput")
    selB_d = nc.dram_tensor("selB", [128, 64], dt.float32,
                            kind="ExternalInput")
    Ad_d = nc.dram_tensor("Ad", [64, TPG], dt.float32, kind="ExternalInput")
    v0_d = nc.dram_tensor("v0", [64, TPG], dt.float32, kind="ExternalInput")
    xp_d = nc.dram_tensor("xp", [T_steps, 64, TPG], dt.float16,
                          kind="ExternalInput")
    out_d = nc.dram_tensor("vs", [T_steps, 64, TPG], dt.float32,
                           kind="ExternalOutput")

    with tile.TileContext(nc) as tc:
        with (
            tc.tile_pool(name="sbuf", bufs=1) as pool,
            tc.tile_pool(name="psum", bufs=2, space="PSUM") as psum_pool,
            tc.tile_pool(name="dram", bufs=1, space="DRAM") as dram_pool,
        ):
            idxA = pool.tile_from(idxA_d[:])
            idxB = pool.tile_from(idxB_d[:])
            bidxA = pool.tile_from(bidxA_d[:])
            bidxB = pool.tile_from(bidxB_d[:])
            selA = pool.tile_from(selA_d[:])
            selB = pool.tile_from(selB_d[:])
            Ad = pool.tile_from(Ad_d[:])
            v = pool.tile_from(v0_d[:])

            table = pool.tile([128, NPAIR, 2], dt.float16)
            bufA = pool.tile([128, CH, 2], dt.float16)
            bufB = pool.tile([128, CH, 2], dt.float16)
            PA = pool.tile([128, CH], dt.float16)
            PB = pool.tile([128, CH], dt.float16)
            X = pool.tile([128, STREAM], dt.float32)
            wqa = [pool.tile([128, 2 * CH], dt.float8e4, name=f"wqa{i}")
                   for i in range(2)]
            wqb = [pool.tile([128, 2 * CH], dt.float8e4, name=f"wqb{i}")
                   for i in range(2)]
            boundA = pool.tile([128, 1 + TPG], dt.float32)
            boundB = pool.tile([128, 1 + TPG], dt.float32)
            D = pool.tile([128, 2 * TPG], dt.float32)
            r_sb = pool.tile([64, TPG], dt.float16)
            xq = [pool.tile([64, TPG], dt.float16, name=f"xq{i}")
                  for i in range(2)]
            t1 = pool.tile([64, TPG], dt.float32)

            r_own = dram_pool.tile([B, CORE_PAD], dt.float16)
            r_all = dram_pool.tile([NC, B * CORE_PAD], dt.float16)

            nc.vector.memset(boundA[:, 0:1], 0.0)
            nc.vector.memset(boundB[:, 0:1], 0.0)
            nc.vector.memset(D[:], 0.0)
            nc.sync.dma_start(xq[0][:], xp_d[0])
            nc.sync.dma_start(wqa[0][:], wqA_d[:, 0: 2 * CH])
            nc.sync.dma_start(wqb[0][:], wqB_d[:, 0: 2 * CH])

            bufAf = bufA[:].rearrange("p c d -> p (c d)")
            bufBf = bufB[:].rearrange("p c d -> p (c d)")

            for t in range(T_steps):
                # ---- halo: r = relu(v) -> r_own -> AllGather -> table ----
                nc.scalar.activation(r_sb[:], v[:], AF.Relu)
                nc.sync.dma_start(
                    r_own[:].rearrange("b (gc t) -> (b gc) t", gc=NGC),
                    r_sb[:],
                )
                nc.gpsimd.collective_compute(
                    "AllGather", OP.bypass,
                    replica_groups=[list(range(NC))],
                    ins=[r_own[:].opt()], outs=[r_all[:].opt()],
                )
                tabf = table[:].rearrange("p c d -> p (c d)")
                for gg in range(NGC):
                    for ss in range(2):
                        dst3 = tabf[16 * gg + 8 * ss: 16 * gg + 8 * ss + 8,
                                    :].rearrange("p (c l) -> p c l", c=4)
                        src3 = r_all[4 * ss: 4 * ss + 4, :].rearrange(
                            "c (b l) -> c b l", b=B).transpose([1, 0, 2])
                        eng = nc.sync if (gg % 2 == 0) else nc.scalar
                        eng.dma_start(dst3, src3)
                if t + 1 < T_steps:
                    nc.scalar.dma_start(xq[(t + 1) % 2][:], xp_d[t + 1])

                # ---- edge phase ----
                for ec in range(NCH):
                    isl = slice(ec * CH // 16, (ec + 1) * CH // 16)
                    nc.gpsimd.ap_gather(bufA[:], table[:], idxA[:, isl],
                                        channels=128, num_elems=NPAIR, d=2,
                                        num_idxs=CH)
                    nc.gpsimd.ap_gather(bufB[:], table[:], idxB[:, isl],
                                        channels=128, num_elems=NPAIR, d=2,
                                        num_idxs=CH)
                    nc.vector.tensor_mul(bufAf, bufAf, wqa[ec % 2][:])
                    nc.vector.tensor_mul(bufBf, bufBf, wqb[ec % 2][:])
                    if ec + 1 < NCH:
                        csl = slice((ec + 1) * 2 * CH, (ec + 2) * 2 * CH)
                        nc.scalar.dma_start(wqa[(ec + 1) % 2][:],
                                            wqA_d[:, csl])
                        nc.scalar.dma_start(wqb[(ec + 1) % 2][:],
                                            wqB_d[:, csl])
                    elif t + 1 < T_steps:
                        nc.scalar.dma_start(wqa[0][:],
                                            wqA_d[:, 0: 2 * CH])
                        nc.scalar.dma_start(wqb[0][:],
                                            wqB_d[:, 0: 2 * CH])
                    nc.vector.tensor_tensor(out=PA[:], in0=bufA[:, :, 0],
                                            in1=bufA[:, :, 1], op=OP.add)
                    nc.vector.tensor_tensor(out=PB[:], in0=bufB[:, :, 0],
                                            in1=bufB[:, :, 1], op=OP.add)
                    xsl = slice(ec * CH, (ec + 1) * CH)
                    init = (0.0 if ec == 0 else
                            X[:, ec * CH - 1: ec * CH])
                    nc.vector.tensor_tensor_scan(X[:, xsl], PA[:], PB[:],
                                                 init, op0=OP.add,
                                                 op1=OP.add)

                # ---- boundaries + diff + reduce ----
                nc.gpsimd.ap_gather(boundA[:, 1: 1 + 784], X[:],
                                    bidxA[:], channels=128,
                                    num_elems=STREAM, d=1, num_idxs=784)
                nc.gpsimd.ap_gather(boundB[:, 1: 1 + 784], X[:],
                                    bidxB[:], channels=128,
                                    num_elems=STREAM, d=1, num_idxs=784)
                nc.vector.tensor_tensor(out=D[:, 0:784],
                                        in0=boundA[:, 1: 1 + 784],
                                        in1=boundA[:, 0:784],
                                        op=OP.subtract)
                nc.vector.tensor_tensor(out=D[:, TPG: TPG + 784],
                                        in0=boundB[:, 1: 1 + 784],
                                        in1=boundB[:, 0:784],
                                        op=OP.subtract)

                H = TPG // 2
                for h in range(2):
                    ps = psum_pool.tile([64, H], dt.float32, space="PSUM",
                                        tag=f"syn{h}")
                    nc.tensor.matmul(ps[:], selA[:],
                                     D[:, h * H: (h + 1) * H],
                                     start=True, stop=False)
                    nc.tensor.matmul(ps[:], selB[:],
                                     D[:, TPG + h * H: TPG + (h + 1) * H],
                                     start=False, stop=True)
                    hsl = slice(h * H, (h + 1) * H)
                    nc.vector.tensor_tensor(t1[:, hsl], ps[:],
                                            xq[t % 2][:, hsl], op=OP.add)
                nc.vector.tensor_mul(v[:], v[:], Ad[:])
                nc.vector.tensor_add(v[:], v[:], t1[:])
                nc.scalar.dma_start(out_d[t], v[:])

    nc.compile()
    return nc


def _get_nc(T_steps):
    key = ("nc2", T_steps)
    if key not in _CACHE:
        _CACHE[key] = _build(T_steps)
    return _CACHE[key]


def _decode(res, T_steps, node_gc, node_tl):
    out = np.empty((T_steps, B, N_NODES), dtype=np.float32)
    for c in range(NC):
        vs = res[c]["vs"]  # [T, 64, TPG], partition (b, gc)
        v4 = vs.reshape(T_steps, B, NGC, TPG)
        nodes = np.arange(c * CORE_REAL, (c + 1) * CORE_REAL)
        out[:, :, nodes] = v4[:, :, node_gc[nodes], node_tl[nodes]]
    return out


def kernel(x, bias, time_const, sign, syn_count, syn_strength,
           source_idx, target_idx):
    from concourse.bass_utils import run_bass_kernel_spmd

    x = np.asarray(x, dtype=np.float32)
    bias = np.asarray(bias, dtype=np.float32)
    time_const = np.asarray(time_const, dtype=np.float32)
    sign = np.asarray(sign, dtype=np.float32)
    syn_count = np.asarray(syn_count, dtype=np.float32)
    syn_strength = np.asarray(syn_strength, dtype=np.float32)
    T_steps = x.shape[0]

    per_core = _preprocess(x, bias, time_const, sign, syn_count,
                           syn_strength, source_idx, target_idx)
    node_gc, node_tl = _CACHE["maps"]
    nc = _get_nc(T_steps)
    t0 = time.perf_counter()
    res = run_bass_kernel_spmd(nc, per_core, core_ids=list(range(NC)))
    t1 = time.perf_counter()
    print(f"[kernel] run_bass_kernel_spmd wall: {t1 - t0:.3f}s",
          file=sys.stderr)
    return _decode(res.results, T_steps, node_gc, node_tl)

